# revision 3
# baseline (speedup 1.0000x reference)
"""TRN2 Bass kernel for nn_MAD_4612794876395 (retrieval_knn) — v2.

Math identical to baseline: with dist = softmax_k(-||pos_d - pos_r||) and
sum_k dist = 1, the reference collapses to
    out[b,c] = wmem@adapt_w + adapt_b + wdiff@field_b.reshape(H,C)
             + sum_h wdiff[b,h] * (date@field_w)[b, h*C+c]
The 137-GFLOP grad term runs on 8 NeuronCores, tensor-parallel over
field_w's 65536 columns (64 h per core).

v2 vs baseline (which was DVE-bound at ~300us):
  - operands pre-converted to bf16 on host (half DMA, FWL weight loads)
  - loop order (slice-quad, b-tile): stationary date chunk reused across
    4 matmuls -> LDWEIGHTS amortized 4x; 8 PSUM banks ping-pong
  - h-contraction: one ACT copy [128,512] PSUM->SBUF per slice, then 4
    SBUF-only DVE scalar_tensor_tensor ops (DVE busy 264us -> 199us)
  - few, large DMAs (4KB rows) emitted in consumption order; wd packed
    on host into one [128, NB*HSH] tile
"""
import sys

sys.path.insert(0, "/opt/trn_rl_repo")

import numpy as np
import ml_dtypes

N_DATA, F, H, C, K, B = 100000, 512, 512, 128, 8, 2048
NCORES = 8
HSH = H // NCORES          # 64 h-values per core
SH = HSH * C               # 8192 field_w cols per core
P = 128
NB = B // P                # 16 b-tiles
NS = SH // 512             # 16 n-slices of 512 cols (4 h each)
NQ = NS // 4               # 4 slice-quads

_NC = None
USE_POOL = False


def _build():
    import concourse.mybir as mybir
    import concourse.tile as tile
    from concourse import bacc

    nc = bacc.Bacc(None, target_bir_lowering=False, debug=False)
    dateT = nc.dram_tensor("dateT", [F, B], mybir.dt.bfloat16, kind="ExternalInput")
    wd = nc.dram_tensor("wd", [P, NB * HSH], mybir.dt.float32, kind="ExternalInput")
    fw = nc.dram_tensor("fw", [F, SH], mybir.dt.bfloat16, kind="ExternalInput")
    partial = nc.dram_tensor("partial", [B, 2 * C], mybir.dt.float32, kind="ExternalOutput")

    with tile.TileContext(nc) as tc:
        with (
            tc.tile_pool(name="const", bufs=1) as cp,
            tc.tile_pool(name="gsp", bufs=2) as gsp,
            tc.tile_pool(name="ps", bufs=2, space="PSUM") as ps,
        ):
            # DMAs in consumption order; first-round operands arrive as
            # small tiles for a fast PE ramp, the rest as big 4KB-row slabs.
            dra = [None] * 4    # dateT[fc], b-columns 0:256 (t 0..1)
            dmid = [None] * 4   # dateT[fc], b-columns 256:1024 (t 2..7)
            drb = [None] * 4    # dateT[fc], b-columns 1024:2048 (t 8..15)
            fw0 = [[None] * 4 for _ in range(4)]   # q0 per-slice tiles [fc][i]
            fwq = [[None] * 4 for _ in range(NQ)]  # q1..q3 slabs [q][fc]
            for fc in range(4):
                dra[fc] = cp.tile([P, 2 * P], mybir.dt.bfloat16,
                                  name=f"dra{fc}")
                nc.sync.dma_start(dra[fc][:], dateT[fc * P:(fc + 1) * P,
                                                    0:2 * P])
                fw0[fc][0] = cp.tile([P, 512], mybir.dt.bfloat16,
                                     name=f"fw0_{fc}_0")
                nc.sync.dma_start(fw0[fc][0][:], fw[fc * P:(fc + 1) * P, 0:512])
            wdp0 = cp.tile([P, 4 * HSH], mybir.dt.float32, name="wdp0")
            nc.sync.dma_start(wdp0[:], wd[:, 0:4 * HSH])
            for i in range(1, 4):
                for fc in range(4):
                    fw0[fc][i] = cp.tile([P, 512], mybir.dt.bfloat16,
                                         name=f"fw0_{fc}_{i}")
                    nc.sync.dma_start(
                        fw0[fc][i][:],
                        fw[fc * P:(fc + 1) * P, i * 512:(i + 1) * 512])
            for fc in range(4):
                dmid[fc] = cp.tile([P, 6 * P], mybir.dt.bfloat16,
                                   name=f"dmid{fc}")
                nc.sync.dma_start(dmid[fc][:], dateT[fc * P:(fc + 1) * P,
                                                     2 * P:B // 2])
            wdp1 = cp.tile([P, 12 * HSH], mybir.dt.float32, name="wdp1")
            nc.sync.dma_start(wdp1[:], wd[:, 4 * HSH:])
            for fc in range(4):
                drb[fc] = cp.tile([P, B // 2], mybir.dt.bfloat16,
                                  name=f"drb{fc}")
                nc.sync.dma_start(drb[fc][:], dateT[fc * P:(fc + 1) * P,
                                                    B // 2:B])
            for q in range(1, NQ):
                for fc in range(4):
                    fwq[q][fc] = cp.tile([P, 4 * 512], mybir.dt.bfloat16,
                                         name=f"fw{q}_{fc}")
                    nc.sync.dma_start(
                        fwq[q][fc][:],
                        fw[fc * P:(fc + 1) * P, q * 2048:(q + 1) * 2048])
            accE, accO = [], []
            for t in range(NB):
                a_t = cp.tile([P, C], mybir.dt.float32, name=f"accE{t}")
                nc.any.memset(a_t[:], 0.0)
                accE.append(a_t)
                b_t = cp.tile([P, C], mybir.dt.float32, name=f"accO{t}")
                nc.any.memset(b_t[:], 0.0)
                accO.append(b_t)

            out_t = []
            for q in range(NQ):
                for t in range(NB):
                    if t < 2:
                        dcol, tc_ = dra, t
                    elif t < 8:
                        dcol, tc_ = dmid, t - 2
                    else:
                        dcol, tc_ = drb, t - 8
                    g = [ps.tile([P, 512], mybir.dt.float32, name="g",
                                 tag=f"g{i}") for i in range(4)]
                    last_round = (q == NQ - 1 and t == NB - 1)
                    # final round slice-major: each bank finishes early so
                    # its copy+STT chain overlaps the remaining matmuls
                    mm_order = ([(fc, i) for i in range(4) for fc in range(4)]
                                if last_round else
                                [(fc, i) for fc in range(4) for i in range(4)])
                    for fc, i in mm_order:
                        rhs = (fw0[fc][i][:] if q == 0 else
                               fwq[q][fc][:, i * 512:(i + 1) * 512])
                        nc.tensor.matmul(
                            g[i][:], dcol[fc][:, tc_ * P:(tc_ + 1) * P],
                            rhs, start=(fc == 0), stop=(fc == 3))
                    for i in range(4):
                        acc = accE[t] if i % 2 == 0 else accO[t]
                        if USE_POOL:
                            # STT straight from PSUM; DVE and Pool each own
                            # an independent accumulator chain
                            src = g[i]
                            eng = nc.vector if i % 2 == 0 else nc.gpsimd
                        else:
                            gs_i = gsp.tile([P, 512], mybir.dt.float32,
                                            name="gs", tag=f"gs{i}")
                            nc.scalar.copy(gs_i[:], g[i][:])
                            src = gs_i
                            eng = nc.vector
                        for l in range(4):
                            if t < 4:
                                wdt, hcol = wdp0, t * HSH + 16 * q + 4 * i + l
                            else:
                                wdt = wdp1
                                hcol = (t - 4) * HSH + 16 * q + 4 * i + l
                            eng.scalar_tensor_tensor(
                                out=acc[:],
                                in0=src[:, l * C:(l + 1) * C],
                                scalar=wdt[:, hcol:hcol + 1],
                                in1=acc[:],
                                op0=mybir.AluOpType.mult,
                                op1=mybir.AluOpType.add,
                            )
                    if q == NQ - 1:
                        # host adds the two halves; E-half DMA overlaps the
                        # O-chain's final STTs
                        nc.sync.dma_start(partial[t * P:(t + 1) * P, 0:C],
                                          accE[t][:])
                        nc.sync.dma_start(partial[t * P:(t + 1) * P, C:2 * C],
                                          accO[t][:])
    nc.finalize()
    return nc


def _host_phase1(idx, date, train_dates, mem, train_nns, pos_w, pos_b,
                 field_b, adapt_w, adapt_b):
    refs = train_nns[idx]                                   # [B, K]
    pos_d = date @ pos_w + pos_b                            # [B, H]
    pos_r = (train_dates[refs.reshape(-1)] @ pos_w + pos_b).reshape(B, K, H)
    diff = pos_d[:, None, :] - pos_r                        # [B, K, H]
    norm = np.sqrt((diff * diff).sum(-1))                   # [B, K]
    m = norm.min(axis=1, keepdims=True)
    e = np.exp(m - norm)
    dist = e / e.sum(axis=1, keepdims=True)                 # [B, K]
    wdiff = np.einsum("bk,bkh->bh", dist, diff).astype(np.float32)
    wmem = np.einsum("bk,bkc->bc", dist, mem[refs]).astype(np.float32)
    const = wmem @ adapt_w + adapt_b + wdiff @ field_b.reshape(H, C)
    return wdiff, const.astype(np.float32)


def kernel(idx, date, train_dates, mem, train_nns, pos_w, pos_b, field_w,
           field_b, adapt_w, adapt_b):
    global _NC
    from concourse.bass_utils import run_bass_kernel_spmd

    idx = np.asarray(idx)
    date = np.asarray(date, dtype=np.float32)
    train_dates = np.asarray(train_dates, dtype=np.float32)
    mem = np.asarray(mem, dtype=np.float32)
    train_nns = np.asarray(train_nns)
    pos_w = np.asarray(pos_w, dtype=np.float32)
    pos_b = np.asarray(pos_b, dtype=np.float32)
    field_w = np.asarray(field_w, dtype=np.float32)
    field_b = np.asarray(field_b, dtype=np.float32)
    adapt_w = np.asarray(adapt_w, dtype=np.float32)
    adapt_b = np.asarray(adapt_b, dtype=np.float32)

    wdiff, const = _host_phase1(idx, date, train_dates, mem, train_nns,
                                pos_w, pos_b, field_b, adapt_w, adapt_b)

    if _NC is None:
        _NC = _build()
    dateT16 = np.ascontiguousarray(date.T).astype(ml_dtypes.bfloat16)
    fw16 = field_w.astype(ml_dtypes.bfloat16)
    in_maps = []
    for i in range(NCORES):
        wds = wdiff[:, i * HSH:(i + 1) * HSH]               # [B, HSH]
        wdp = np.ascontiguousarray(
            wds.reshape(NB, P, HSH).transpose(1, 0, 2).reshape(P, NB * HSH))
        in_maps.append({
            "dateT": dateT16,
            "wd": wdp,
            "fw": np.ascontiguousarray(fw16[:, i * SH:(i + 1) * SH]),
        })
    res = run_bass_kernel_spmd(_NC, in_maps, core_ids=list(range(NCORES)))
    grad_term = np.zeros((B, C), dtype=np.float32)
    for i in range(NCORES):
        p = res.results[i]["partial"]
        grad_term += p[:, :C] + p[:, C:]
    return (const + grad_term).astype(np.float32)


# revision 4
# speedup vs baseline: 1.0030x; 1.0030x over previous
"""TRN2 Bass kernel for nn_MAD_4612794876395 (retrieval_knn) — v2.

Math identical to baseline: with dist = softmax_k(-||pos_d - pos_r||) and
sum_k dist = 1, the reference collapses to
    out[b,c] = wmem@adapt_w + adapt_b + wdiff@field_b.reshape(H,C)
             + sum_h wdiff[b,h] * (date@field_w)[b, h*C+c]
The 137-GFLOP grad term runs on 8 NeuronCores, tensor-parallel over
field_w's 65536 columns (64 h per core).

v2 vs baseline (which was DVE-bound at ~300us):
  - operands pre-converted to bf16 on host (half DMA, FWL weight loads)
  - loop order (slice-quad, b-tile): stationary date chunk reused across
    4 matmuls -> LDWEIGHTS amortized 4x; 8 PSUM banks ping-pong
  - h-contraction: one ACT copy [128,512] PSUM->SBUF per slice, then 4
    SBUF-only DVE scalar_tensor_tensor ops (DVE busy 264us -> 200us) into
    two independent accumulator chains (even/odd slices); host adds halves
  - few, large DMAs (4KB rows) emitted in consumption order; wd packed
    on host into one [128, NB*HSH] tile
"""
import sys

sys.path.insert(0, "/opt/trn_rl_repo")

import numpy as np
import ml_dtypes

N_DATA, F, H, C, K, B = 100000, 512, 512, 128, 8, 2048
NCORES = 8
HSH = H // NCORES          # 64 h-values per core
SH = HSH * C               # 8192 field_w cols per core
P = 128
NB = B // P                # 16 b-tiles
NS = SH // 512             # 16 n-slices of 512 cols (4 h each)
NQ = NS // 4               # 4 slice-quads

_NC = None


def _build():
    import concourse.mybir as mybir
    import concourse.tile as tile
    from concourse import bacc

    nc = bacc.Bacc(None, target_bir_lowering=False, debug=False)
    dateT = nc.dram_tensor("dateT", [F, B], mybir.dt.bfloat16, kind="ExternalInput")
    wd = nc.dram_tensor("wd", [P, NB * HSH], mybir.dt.float32, kind="ExternalInput")
    fw = nc.dram_tensor("fw", [F, SH], mybir.dt.bfloat16, kind="ExternalInput")
    partial = nc.dram_tensor("partial", [B, 2 * C], mybir.dt.float32, kind="ExternalOutput")

    with tile.TileContext(nc) as tc:
        with (
            tc.tile_pool(name="const", bufs=1) as cp,
            tc.tile_pool(name="gsp", bufs=2) as gsp,
            tc.tile_pool(name="ps", bufs=2, space="PSUM") as ps,
        ):
            # DMAs in consumption order; first-round operands arrive as
            # small tiles for a fast PE ramp, the rest as big 4KB-row slabs.
            dra = [None] * 4    # dateT[fc], b-columns 0:256 (t 0..1)
            dmid = [None] * 4   # dateT[fc], b-columns 256:1024 (t 2..7)
            drb = [None] * 4    # dateT[fc], b-columns 1024:2048 (t 8..15)
            fw0 = [[None] * 4 for _ in range(4)]   # q0 per-slice tiles [fc][i]
            fwq = [[None] * 4 for _ in range(NQ)]  # q1..q3 slabs [q][fc]
            for fc in range(4):
                dra[fc] = cp.tile([P, 2 * P], mybir.dt.bfloat16,
                                  name=f"dra{fc}")
                nc.sync.dma_start(dra[fc][:], dateT[fc * P:(fc + 1) * P,
                                                    0:2 * P])
                fw0[fc][0] = cp.tile([P, 512], mybir.dt.bfloat16,
                                     name=f"fw0_{fc}_0")
                nc.sync.dma_start(fw0[fc][0][:], fw[fc * P:(fc + 1) * P, 0:512])
            wdp0 = cp.tile([P, 4 * HSH], mybir.dt.float32, name="wdp0")
            nc.sync.dma_start(wdp0[:], wd[:, 0:4 * HSH])
            for i in range(1, 4):
                for fc in range(4):
                    fw0[fc][i] = cp.tile([P, 512], mybir.dt.bfloat16,
                                         name=f"fw0_{fc}_{i}")
                    nc.sync.dma_start(
                        fw0[fc][i][:],
                        fw[fc * P:(fc + 1) * P, i * 512:(i + 1) * 512])
            for fc in range(4):
                dmid[fc] = cp.tile([P, 6 * P], mybir.dt.bfloat16,
                                   name=f"dmid{fc}")
                nc.sync.dma_start(dmid[fc][:], dateT[fc * P:(fc + 1) * P,
                                                     2 * P:B // 2])
            wdp1 = cp.tile([P, 12 * HSH], mybir.dt.float32, name="wdp1")
            nc.sync.dma_start(wdp1[:], wd[:, 4 * HSH:])
            for fc in range(4):
                drb[fc] = cp.tile([P, B // 2], mybir.dt.bfloat16,
                                  name=f"drb{fc}")
                nc.sync.dma_start(drb[fc][:], dateT[fc * P:(fc + 1) * P,
                                                    B // 2:B])
            for q in range(1, NQ):
                for fc in range(4):
                    fwq[q][fc] = cp.tile([P, 4 * 512], mybir.dt.bfloat16,
                                         name=f"fw{q}_{fc}")
                    nc.sync.dma_start(
                        fwq[q][fc][:],
                        fw[fc * P:(fc + 1) * P, q * 2048:(q + 1) * 2048])
            accE, accO = [], []
            for t in range(NB):
                a_t = cp.tile([P, C], mybir.dt.float32, name=f"accE{t}")
                nc.any.memset(a_t[:], 0.0)
                accE.append(a_t)
                b_t = cp.tile([P, C], mybir.dt.float32, name=f"accO{t}")
                nc.any.memset(b_t[:], 0.0)
                accO.append(b_t)

            for q in range(NQ):
                for t in range(NB):
                    if t < 2:
                        dcol, tc_ = dra, t
                    elif t < 8:
                        dcol, tc_ = dmid, t - 2
                    else:
                        dcol, tc_ = drb, t - 8
                    g = [ps.tile([P, 512], mybir.dt.float32, name="g",
                                 tag=f"g{i}") for i in range(4)]
                    last_round = (q == NQ - 1 and t == NB - 1)
                    # final round slice-major: each bank finishes early so
                    # its copy+STT chain overlaps the remaining matmuls
                    mm_order = ([(fc, i) for i in range(4) for fc in range(4)]
                                if last_round else
                                [(fc, i) for fc in range(4) for i in range(4)])
                    for fc, i in mm_order:
                        rhs = (fw0[fc][i][:] if q == 0 else
                               fwq[q][fc][:, i * 512:(i + 1) * 512])
                        nc.tensor.matmul(
                            g[i][:], dcol[fc][:, tc_ * P:(tc_ + 1) * P],
                            rhs, start=(fc == 0), stop=(fc == 3))
                    for i in range(4):
                        acc = accE[t] if i % 2 == 0 else accO[t]
                        gs_i = gsp.tile([P, 512], mybir.dt.float32,
                                        name="gs", tag=f"gs{i}")
                        nc.scalar.copy(gs_i[:], g[i][:])
                        for l in range(4):
                            if t < 4:
                                wdt, hcol = wdp0, t * HSH + 16 * q + 4 * i + l
                            else:
                                wdt = wdp1
                                hcol = (t - 4) * HSH + 16 * q + 4 * i + l
                            nc.vector.scalar_tensor_tensor(
                                out=acc[:],
                                in0=gs_i[:, l * C:(l + 1) * C],
                                scalar=wdt[:, hcol:hcol + 1],
                                in1=acc[:],
                                op0=mybir.AluOpType.mult,
                                op1=mybir.AluOpType.add,
                            )
                    if q == NQ - 1:
                        # host adds the two halves; E-half DMA overlaps the
                        # O-chain's final STTs
                        nc.sync.dma_start(partial[t * P:(t + 1) * P, 0:C],
                                          accE[t][:])
                        nc.sync.dma_start(partial[t * P:(t + 1) * P, C:2 * C],
                                          accO[t][:])
    nc.finalize()
    return nc


def _host_phase1(idx, date, train_dates, mem, train_nns, pos_w, pos_b,
                 field_b, adapt_w, adapt_b):
    refs = train_nns[idx]                                   # [B, K]
    pos_d = date @ pos_w + pos_b                            # [B, H]
    pos_r = (train_dates[refs.reshape(-1)] @ pos_w + pos_b).reshape(B, K, H)
    diff = pos_d[:, None, :] - pos_r                        # [B, K, H]
    norm = np.sqrt((diff * diff).sum(-1))                   # [B, K]
    m = norm.min(axis=1, keepdims=True)
    e = np.exp(m - norm)
    dist = e / e.sum(axis=1, keepdims=True)                 # [B, K]
    wdiff = np.einsum("bk,bkh->bh", dist, diff).astype(np.float32)
    wmem = np.einsum("bk,bkc->bc", dist, mem[refs]).astype(np.float32)
    const = wmem @ adapt_w + adapt_b + wdiff @ field_b.reshape(H, C)
    return wdiff, const.astype(np.float32)


def kernel(idx, date, train_dates, mem, train_nns, pos_w, pos_b, field_w,
           field_b, adapt_w, adapt_b):
    global _NC
    from concourse.bass_utils import run_bass_kernel_spmd

    idx = np.asarray(idx)
    date = np.asarray(date, dtype=np.float32)
    train_dates = np.asarray(train_dates, dtype=np.float32)
    mem = np.asarray(mem, dtype=np.float32)
    train_nns = np.asarray(train_nns)
    pos_w = np.asarray(pos_w, dtype=np.float32)
    pos_b = np.asarray(pos_b, dtype=np.float32)
    field_w = np.asarray(field_w, dtype=np.float32)
    field_b = np.asarray(field_b, dtype=np.float32)
    adapt_w = np.asarray(adapt_w, dtype=np.float32)
    adapt_b = np.asarray(adapt_b, dtype=np.float32)

    wdiff, const = _host_phase1(idx, date, train_dates, mem, train_nns,
                                pos_w, pos_b, field_b, adapt_w, adapt_b)

    if _NC is None:
        _NC = _build()
    dateT16 = np.ascontiguousarray(date.T).astype(ml_dtypes.bfloat16)
    fw16 = field_w.astype(ml_dtypes.bfloat16)
    in_maps = []
    for i in range(NCORES):
        wds = wdiff[:, i * HSH:(i + 1) * HSH]               # [B, HSH]
        wdp = np.ascontiguousarray(
            wds.reshape(NB, P, HSH).transpose(1, 0, 2).reshape(P, NB * HSH))
        in_maps.append({
            "dateT": dateT16,
            "wd": wdp,
            "fw": np.ascontiguousarray(fw16[:, i * SH:(i + 1) * SH]),
        })
    res = run_bass_kernel_spmd(_NC, in_maps, core_ids=list(range(NCORES)))
    grad_term = np.zeros((B, C), dtype=np.float32)
    for i in range(NCORES):
        p = res.results[i]["partial"]
        grad_term += p[:, :C] + p[:, C:]
    return (const + grad_term).astype(np.float32)


# revision 5
# speedup vs baseline: 1.0110x; 1.0079x over previous
"""TRN2 Bass kernel for nn_MAD_4612794876395 (retrieval_knn) — v2.

Math identical to baseline: with dist = softmax_k(-||pos_d - pos_r||) and
sum_k dist = 1, the reference collapses to
    out[b,c] = wmem@adapt_w + adapt_b + wdiff@field_b.reshape(H,C)
             + sum_h wdiff[b,h] * (date@field_w)[b, h*C+c]
The 137-GFLOP grad term runs on 8 NeuronCores, tensor-parallel over
field_w's 65536 columns (64 h per core).

v2 vs baseline (which was DVE-bound at ~300us):
  - operands pre-converted to bf16 on host (half DMA, FWL weight loads)
  - loop order (slice-quad, b-tile): stationary date chunk reused across
    4 matmuls -> LDWEIGHTS amortized 4x; 8 PSUM banks ping-pong
  - h-contraction: one ACT copy [128,512] PSUM->SBUF per slice, then 4
    SBUF-only DVE scalar_tensor_tensor ops (DVE busy 264us -> 200us) into
    two independent accumulator chains (even/odd slices); host adds halves
  - few, large DMAs (4KB rows) emitted in consumption order; wd packed
    on host into one [128, NB*HSH] tile
"""
import sys

sys.path.insert(0, "/opt/trn_rl_repo")

import numpy as np
import ml_dtypes

N_DATA, F, H, C, K, B = 100000, 512, 512, 128, 8, 2048
NCORES = 8
HSH = H // NCORES          # 64 h-values per core
SH = HSH * C               # 8192 field_w cols per core
P = 128
NB = B // P                # 16 b-tiles
NS = SH // 512             # 16 n-slices of 512 cols (4 h each)
NQ = NS // 4               # 4 slice-quads

_NC = None


def _build():
    import concourse.mybir as mybir
    import concourse.tile as tile
    from concourse import bacc

    nc = bacc.Bacc(None, target_bir_lowering=False, debug=False)
    dateT = nc.dram_tensor("dateT", [F, B], mybir.dt.bfloat16, kind="ExternalInput")
    wd = nc.dram_tensor("wd", [P, NB * HSH], mybir.dt.float32, kind="ExternalInput")
    fw = nc.dram_tensor("fw", [F, SH], mybir.dt.bfloat16, kind="ExternalInput")
    partial = nc.dram_tensor("partial", [B, 2 * C], mybir.dt.float32, kind="ExternalOutput")

    with tile.TileContext(nc) as tc:
        with (
            tc.tile_pool(name="const", bufs=1) as cp,
            tc.tile_pool(name="gsp", bufs=2) as gsp,
            tc.tile_pool(name="ps", bufs=2, space="PSUM") as ps,
        ):
            # DMAs in consumption order; first-round operands arrive as
            # small tiles for a fast PE ramp, the rest as big 4KB-row slabs.
            dra = [None] * 4    # dateT[fc], b-columns 0:256 (t 0..1)
            dmid = [None] * 4   # dateT[fc], b-columns 256:1024 (t 2..7)
            drb = [None] * 4    # dateT[fc], b-columns 1024:2048 (t 8..15)
            fw0 = [[None] * 4 for _ in range(4)]   # q0 per-slice tiles [fc][i]
            fwq = [[None] * 4 for _ in range(NQ)]  # q1..q3 slabs [q][fc]
            # first-round operands issue on both HWDGE queues (SP + ACT)
            # so the serial per-DMA issue cost doesn't gate the PE ramp
            for fc in range(4):
                dra[fc] = cp.tile([P, 2 * P], mybir.dt.bfloat16,
                                  name=f"dra{fc}")
                nc.sync.dma_start(dra[fc][:], dateT[fc * P:(fc + 1) * P,
                                                    0:2 * P])
                fw0[fc][0] = cp.tile([P, 512], mybir.dt.bfloat16,
                                     name=f"fw0_{fc}_0")
                nc.scalar.dma_start(fw0[fc][0][:],
                                    fw[fc * P:(fc + 1) * P, 0:512])
            wdp0 = cp.tile([P, 4 * HSH], mybir.dt.float32, name="wdp0")
            nc.scalar.dma_start(wdp0[:], wd[:, 0:4 * HSH])
            for i in range(1, 4):
                for fc in range(4):
                    fw0[fc][i] = cp.tile([P, 512], mybir.dt.bfloat16,
                                         name=f"fw0_{fc}_{i}")
                    nc.sync.dma_start(
                        fw0[fc][i][:],
                        fw[fc * P:(fc + 1) * P, i * 512:(i + 1) * 512])
            for fc in range(4):
                dmid[fc] = cp.tile([P, 6 * P], mybir.dt.bfloat16,
                                   name=f"dmid{fc}")
                nc.sync.dma_start(dmid[fc][:], dateT[fc * P:(fc + 1) * P,
                                                     2 * P:B // 2])
            wdp1 = cp.tile([P, 12 * HSH], mybir.dt.float32, name="wdp1")
            nc.sync.dma_start(wdp1[:], wd[:, 4 * HSH:])
            for fc in range(4):
                drb[fc] = cp.tile([P, B // 2], mybir.dt.bfloat16,
                                  name=f"drb{fc}")
                nc.sync.dma_start(drb[fc][:], dateT[fc * P:(fc + 1) * P,
                                                    B // 2:B])
            for q in range(1, NQ):
                for fc in range(4):
                    fwq[q][fc] = cp.tile([P, 4 * 512], mybir.dt.bfloat16,
                                         name=f"fw{q}_{fc}")
                    nc.sync.dma_start(
                        fwq[q][fc][:],
                        fw[fc * P:(fc + 1) * P, q * 2048:(q + 1) * 2048])
            accE, accO = [], []
            for t in range(NB):
                a_t = cp.tile([P, C], mybir.dt.float32, name=f"accE{t}")
                nc.any.memset(a_t[:], 0.0)
                accE.append(a_t)
                b_t = cp.tile([P, C], mybir.dt.float32, name=f"accO{t}")
                nc.any.memset(b_t[:], 0.0)
                accO.append(b_t)

            for q in range(NQ):
                for t in range(NB):
                    if t < 2:
                        dcol, tc_ = dra, t
                    elif t < 8:
                        dcol, tc_ = dmid, t - 2
                    else:
                        dcol, tc_ = drb, t - 8
                    g = [ps.tile([P, 512], mybir.dt.float32, name="g",
                                 tag=f"g{i}") for i in range(4)]
                    last_round = (q == NQ - 1 and t == NB - 1)
                    # final round slice-major: each bank finishes early so
                    # its copy+STT chain overlaps the remaining matmuls
                    mm_order = ([(fc, i) for i in range(4) for fc in range(4)]
                                if last_round else
                                [(fc, i) for fc in range(4) for i in range(4)])
                    for fc, i in mm_order:
                        rhs = (fw0[fc][i][:] if q == 0 else
                               fwq[q][fc][:, i * 512:(i + 1) * 512])
                        nc.tensor.matmul(
                            g[i][:], dcol[fc][:, tc_ * P:(tc_ + 1) * P],
                            rhs, start=(fc == 0), stop=(fc == 3))
                    for i in range(4):
                        acc = accE[t] if i % 2 == 0 else accO[t]
                        gs_i = gsp.tile([P, 512], mybir.dt.float32,
                                        name="gs", tag=f"gs{i}")
                        nc.scalar.copy(gs_i[:], g[i][:])
                        for l in range(4):
                            if t < 4:
                                wdt, hcol = wdp0, t * HSH + 16 * q + 4 * i + l
                            else:
                                wdt = wdp1
                                hcol = (t - 4) * HSH + 16 * q + 4 * i + l
                            nc.vector.scalar_tensor_tensor(
                                out=acc[:],
                                in0=gs_i[:, l * C:(l + 1) * C],
                                scalar=wdt[:, hcol:hcol + 1],
                                in1=acc[:],
                                op0=mybir.AluOpType.mult,
                                op1=mybir.AluOpType.add,
                            )
                    if q == NQ - 1:
                        # host adds the two halves; E-half DMA overlaps the
                        # O-chain's final STTs
                        nc.sync.dma_start(partial[t * P:(t + 1) * P, 0:C],
                                          accE[t][:])
                        nc.sync.dma_start(partial[t * P:(t + 1) * P, C:2 * C],
                                          accO[t][:])
    nc.finalize()
    return nc


def _host_phase1(idx, date, train_dates, mem, train_nns, pos_w, pos_b,
                 field_b, adapt_w, adapt_b):
    refs = train_nns[idx]                                   # [B, K]
    pos_d = date @ pos_w + pos_b                            # [B, H]
    pos_r = (train_dates[refs.reshape(-1)] @ pos_w + pos_b).reshape(B, K, H)
    diff = pos_d[:, None, :] - pos_r                        # [B, K, H]
    norm = np.sqrt((diff * diff).sum(-1))                   # [B, K]
    m = norm.min(axis=1, keepdims=True)
    e = np.exp(m - norm)
    dist = e / e.sum(axis=1, keepdims=True)                 # [B, K]
    wdiff = np.einsum("bk,bkh->bh", dist, diff).astype(np.float32)
    wmem = np.einsum("bk,bkc->bc", dist, mem[refs]).astype(np.float32)
    const = wmem @ adapt_w + adapt_b + wdiff @ field_b.reshape(H, C)
    return wdiff, const.astype(np.float32)


def kernel(idx, date, train_dates, mem, train_nns, pos_w, pos_b, field_w,
           field_b, adapt_w, adapt_b):
    global _NC
    from concourse.bass_utils import run_bass_kernel_spmd

    idx = np.asarray(idx)
    date = np.asarray(date, dtype=np.float32)
    train_dates = np.asarray(train_dates, dtype=np.float32)
    mem = np.asarray(mem, dtype=np.float32)
    train_nns = np.asarray(train_nns)
    pos_w = np.asarray(pos_w, dtype=np.float32)
    pos_b = np.asarray(pos_b, dtype=np.float32)
    field_w = np.asarray(field_w, dtype=np.float32)
    field_b = np.asarray(field_b, dtype=np.float32)
    adapt_w = np.asarray(adapt_w, dtype=np.float32)
    adapt_b = np.asarray(adapt_b, dtype=np.float32)

    wdiff, const = _host_phase1(idx, date, train_dates, mem, train_nns,
                                pos_w, pos_b, field_b, adapt_w, adapt_b)

    if _NC is None:
        _NC = _build()
    dateT16 = np.ascontiguousarray(date.T).astype(ml_dtypes.bfloat16)
    fw16 = field_w.astype(ml_dtypes.bfloat16)
    in_maps = []
    for i in range(NCORES):
        wds = wdiff[:, i * HSH:(i + 1) * HSH]               # [B, HSH]
        wdp = np.ascontiguousarray(
            wds.reshape(NB, P, HSH).transpose(1, 0, 2).reshape(P, NB * HSH))
        in_maps.append({
            "dateT": dateT16,
            "wd": wdp,
            "fw": np.ascontiguousarray(fw16[:, i * SH:(i + 1) * SH]),
        })
    res = run_bass_kernel_spmd(_NC, in_maps, core_ids=list(range(NCORES)))
    grad_term = np.zeros((B, C), dtype=np.float32)
    for i in range(NCORES):
        p = res.results[i]["partial"]
        grad_term += p[:, :C] + p[:, C:]
    return (const + grad_term).astype(np.float32)


# revision 6
# speedup vs baseline: 1.0164x; 1.0054x over previous
"""TRN2 Bass kernel for nn_MAD_4612794876395 (retrieval_knn) — v2.

Math identical to baseline: with dist = softmax_k(-||pos_d - pos_r||) and
sum_k dist = 1, the reference collapses to
    out[b,c] = wmem@adapt_w + adapt_b + wdiff@field_b.reshape(H,C)
             + sum_h wdiff[b,h] * (date@field_w)[b, h*C+c]
The 137-GFLOP grad term runs on 8 NeuronCores, tensor-parallel over
field_w's 65536 columns (64 h per core).

v2 vs baseline (which was DVE-bound at ~300us):
  - operands pre-converted to bf16 on host (half DMA, FWL weight loads)
  - loop order (slice-quad, b-tile): stationary date chunk reused across
    4 matmuls -> LDWEIGHTS amortized 4x; 8 PSUM banks ping-pong
  - h-contraction: one ACT copy [128,512] PSUM->SBUF per slice, then 4
    SBUF-only DVE scalar_tensor_tensor ops (DVE busy 264us -> 200us) into
    two independent accumulator chains (even/odd slices); host adds halves
  - few, large DMAs (4KB rows) emitted in consumption order; wd packed
    on host into one [128, NB*HSH] tile
"""
import sys

sys.path.insert(0, "/opt/trn_rl_repo")

import numpy as np
import ml_dtypes

N_DATA, F, H, C, K, B = 100000, 512, 512, 128, 8, 2048
NCORES = 8
HSH = H // NCORES          # 64 h-values per core
SH = HSH * C               # 8192 field_w cols per core
P = 128
NB = B // P                # 16 b-tiles
NS = SH // 512             # 16 n-slices of 512 cols (4 h each)
NQ = NS // 4               # 4 slice-quads

_NC = None


def _build():
    import concourse.mybir as mybir
    import concourse.tile as tile
    from concourse import bacc

    nc = bacc.Bacc(None, target_bir_lowering=False, debug=False)
    dateT = nc.dram_tensor("dateT", [F, B], mybir.dt.bfloat16, kind="ExternalInput")
    wd = nc.dram_tensor("wd", [P, NB * HSH], mybir.dt.float32, kind="ExternalInput")
    fw = nc.dram_tensor("fw", [F, SH], mybir.dt.bfloat16, kind="ExternalInput")
    partial = nc.dram_tensor("partial", [B, 2 * C], mybir.dt.float32, kind="ExternalOutput")

    with tile.TileContext(nc) as tc:
        with (
            tc.tile_pool(name="const", bufs=1) as cp,
            tc.tile_pool(name="gsp", bufs=2) as gsp,
            tc.tile_pool(name="ps", bufs=2, space="PSUM") as ps,
        ):
            # DMAs in consumption order; first-round operands arrive as
            # small tiles for a fast PE ramp, the rest as big 4KB-row slabs.
            dra = [None] * 4    # dateT[fc], b-columns 0:256 (t 0..1)
            dmid = [None] * 4   # dateT[fc], b-columns 256:1024 (t 2..7)
            drb = [None] * 4    # dateT[fc], b-columns 1024:2048 (t 8..15)
            fw0 = [[None] * 4 for _ in range(4)]   # q0 per-slice tiles [fc][i]
            fwq = [[None] * 4 for _ in range(NQ)]  # q1..q3 slabs [q][fc]
            # first-round operands issue on both HWDGE queues (SP + ACT)
            # so the serial per-DMA issue cost doesn't gate the PE ramp
            for fc in range(4):
                dra[fc] = cp.tile([P, 2 * P], mybir.dt.bfloat16,
                                  name=f"dra{fc}")
                nc.sync.dma_start(dra[fc][:], dateT[fc * P:(fc + 1) * P,
                                                    0:2 * P])
                fw0[fc][0] = cp.tile([P, 512], mybir.dt.bfloat16,
                                     name=f"fw0_{fc}_0")
                nc.scalar.dma_start(fw0[fc][0][:],
                                    fw[fc * P:(fc + 1) * P, 0:512])
            wdp0 = cp.tile([P, 4 * HSH], mybir.dt.float32, name="wdp0")
            nc.scalar.dma_start(wdp0[:], wd[:, 0:4 * HSH])
            for i in range(1, 4):
                for fc in range(4):
                    fw0[fc][i] = cp.tile([P, 512], mybir.dt.bfloat16,
                                         name=f"fw0_{fc}_{i}")
                    nc.sync.dma_start(
                        fw0[fc][i][:],
                        fw[fc * P:(fc + 1) * P, i * 512:(i + 1) * 512])
            for fc in range(4):
                dmid[fc] = cp.tile([P, 6 * P], mybir.dt.bfloat16,
                                   name=f"dmid{fc}")
                nc.sync.dma_start(dmid[fc][:], dateT[fc * P:(fc + 1) * P,
                                                     2 * P:B // 2])
            wdp1 = cp.tile([P, 12 * HSH], mybir.dt.float32, name="wdp1")
            nc.sync.dma_start(wdp1[:], wd[:, 4 * HSH:])
            for fc in range(4):
                drb[fc] = cp.tile([P, B // 2], mybir.dt.bfloat16,
                                  name=f"drb{fc}")
                nc.sync.dma_start(drb[fc][:], dateT[fc * P:(fc + 1) * P,
                                                    B // 2:B])
            for q in range(1, NQ):
                for fc in range(4):
                    fwq[q][fc] = cp.tile([P, 4 * 512], mybir.dt.bfloat16,
                                         name=f"fw{q}_{fc}")
                    nc.sync.dma_start(
                        fwq[q][fc][:],
                        fw[fc * P:(fc + 1) * P, q * 2048:(q + 1) * 2048])
            accE, accO = [], []
            for t in range(NB):
                a_t = cp.tile([P, C], mybir.dt.float32, name=f"accE{t}")
                nc.any.memset(a_t[:], 0.0)
                accE.append(a_t)
                b_t = cp.tile([P, C], mybir.dt.float32, name=f"accO{t}")
                nc.any.memset(b_t[:], 0.0)
                accO.append(b_t)

            for q in range(NQ):
                for t in range(NB):
                    if t < 2:
                        dcol, tc_ = dra, t
                    elif t < 8:
                        dcol, tc_ = dmid, t - 2
                    else:
                        dcol, tc_ = drb, t - 8
                    g = [ps.tile([P, 512], mybir.dt.float32, name="g",
                                 tag=f"g{i}") for i in range(4)]
                    # slice-major: each PSUM bank's accumulation completes
                    # early in the round, so its copy+STT chain overlaps the
                    # remaining matmuls instead of spilling into the next
                    # round (LDWEIGHTS is emitted per-matmul either way)
                    mm_order = [(fc, i) for i in range(4) for fc in range(4)]
                    for fc, i in mm_order:
                        rhs = (fw0[fc][i][:] if q == 0 else
                               fwq[q][fc][:, i * 512:(i + 1) * 512])
                        nc.tensor.matmul(
                            g[i][:], dcol[fc][:, tc_ * P:(tc_ + 1) * P],
                            rhs, start=(fc == 0), stop=(fc == 3))
                    for i in range(4):
                        acc = accE[t] if i % 2 == 0 else accO[t]
                        gs_i = gsp.tile([P, 512], mybir.dt.float32,
                                        name="gs", tag=f"gs{i}")
                        nc.scalar.copy(gs_i[:], g[i][:])
                        for l in range(4):
                            if t < 4:
                                wdt, hcol = wdp0, t * HSH + 16 * q + 4 * i + l
                            else:
                                wdt = wdp1
                                hcol = (t - 4) * HSH + 16 * q + 4 * i + l
                            nc.vector.scalar_tensor_tensor(
                                out=acc[:],
                                in0=gs_i[:, l * C:(l + 1) * C],
                                scalar=wdt[:, hcol:hcol + 1],
                                in1=acc[:],
                                op0=mybir.AluOpType.mult,
                                op1=mybir.AluOpType.add,
                            )
                    if q == NQ - 1:
                        # host adds the two halves; E-half DMA overlaps the
                        # O-chain's final STTs
                        nc.sync.dma_start(partial[t * P:(t + 1) * P, 0:C],
                                          accE[t][:])
                        nc.sync.dma_start(partial[t * P:(t + 1) * P, C:2 * C],
                                          accO[t][:])
    nc.finalize()
    return nc


def _host_phase1(idx, date, train_dates, mem, train_nns, pos_w, pos_b,
                 field_b, adapt_w, adapt_b):
    refs = train_nns[idx]                                   # [B, K]
    pos_d = date @ pos_w + pos_b                            # [B, H]
    pos_r = (train_dates[refs.reshape(-1)] @ pos_w + pos_b).reshape(B, K, H)
    diff = pos_d[:, None, :] - pos_r                        # [B, K, H]
    norm = np.sqrt((diff * diff).sum(-1))                   # [B, K]
    m = norm.min(axis=1, keepdims=True)
    e = np.exp(m - norm)
    dist = e / e.sum(axis=1, keepdims=True)                 # [B, K]
    wdiff = np.einsum("bk,bkh->bh", dist, diff).astype(np.float32)
    wmem = np.einsum("bk,bkc->bc", dist, mem[refs]).astype(np.float32)
    const = wmem @ adapt_w + adapt_b + wdiff @ field_b.reshape(H, C)
    return wdiff, const.astype(np.float32)


def kernel(idx, date, train_dates, mem, train_nns, pos_w, pos_b, field_w,
           field_b, adapt_w, adapt_b):
    global _NC
    from concourse.bass_utils import run_bass_kernel_spmd

    idx = np.asarray(idx)
    date = np.asarray(date, dtype=np.float32)
    train_dates = np.asarray(train_dates, dtype=np.float32)
    mem = np.asarray(mem, dtype=np.float32)
    train_nns = np.asarray(train_nns)
    pos_w = np.asarray(pos_w, dtype=np.float32)
    pos_b = np.asarray(pos_b, dtype=np.float32)
    field_w = np.asarray(field_w, dtype=np.float32)
    field_b = np.asarray(field_b, dtype=np.float32)
    adapt_w = np.asarray(adapt_w, dtype=np.float32)
    adapt_b = np.asarray(adapt_b, dtype=np.float32)

    wdiff, const = _host_phase1(idx, date, train_dates, mem, train_nns,
                                pos_w, pos_b, field_b, adapt_w, adapt_b)

    if _NC is None:
        _NC = _build()
    dateT16 = np.ascontiguousarray(date.T).astype(ml_dtypes.bfloat16)
    fw16 = field_w.astype(ml_dtypes.bfloat16)
    in_maps = []
    for i in range(NCORES):
        wds = wdiff[:, i * HSH:(i + 1) * HSH]               # [B, HSH]
        wdp = np.ascontiguousarray(
            wds.reshape(NB, P, HSH).transpose(1, 0, 2).reshape(P, NB * HSH))
        in_maps.append({
            "dateT": dateT16,
            "wd": wdp,
            "fw": np.ascontiguousarray(fw16[:, i * SH:(i + 1) * SH]),
        })
    res = run_bass_kernel_spmd(_NC, in_maps, core_ids=list(range(NCORES)))
    grad_term = np.zeros((B, C), dtype=np.float32)
    for i in range(NCORES):
        p = res.results[i]["partial"]
        grad_term += p[:, :C] + p[:, C:]
    return (const + grad_term).astype(np.float32)


# revision 7
# speedup vs baseline: 1.0379x; 1.0211x over previous
"""TRN2 Bass kernel for nn_MAD_4612794876395 (retrieval_knn) — v2.

Math identical to baseline: with dist = softmax_k(-||pos_d - pos_r||) and
sum_k dist = 1, the reference collapses to
    out[b,c] = wmem@adapt_w + adapt_b + wdiff@field_b.reshape(H,C)
             + sum_h wdiff[b,h] * (date@field_w)[b, h*C+c]
The 137-GFLOP grad term runs on 8 NeuronCores, tensor-parallel over
field_w's 65536 columns (64 h per core).

v2 vs baseline (which was DVE-bound at ~300us):
  - operands pre-converted to bf16 on host (half DMA, FWL weight loads)
  - loop order (slice-quad, b-tile): stationary date chunk reused across
    4 matmuls -> LDWEIGHTS amortized 4x; 8 PSUM banks ping-pong
  - h-contraction: one ACT copy [128,512] PSUM->SBUF per slice, then 4
    SBUF-only DVE scalar_tensor_tensor ops (DVE busy 264us -> 200us) into
    two independent accumulator chains (even/odd slices); host adds halves
  - few, large DMAs (4KB rows) emitted in consumption order; wd packed
    on host into one [128, NB*HSH] tile
"""
import sys

sys.path.insert(0, "/opt/trn_rl_repo")

import numpy as np
import ml_dtypes

N_DATA, F, H, C, K, B = 100000, 512, 512, 128, 8, 2048
NCORES = 8
HSH = H // NCORES          # 64 h-values per core
SH = HSH * C               # 8192 field_w cols per core
P = 128
NB = B // P                # 16 b-tiles
NS = SH // 512             # 16 n-slices of 512 cols (4 h each)
NQ = NS // 4               # 4 slice-quads

_NC = None


def _build():
    import concourse.mybir as mybir
    import concourse.tile as tile
    from concourse import bacc

    nc = bacc.Bacc(None, target_bir_lowering=False, debug=False)
    dateT = nc.dram_tensor("dateT", [F, B], mybir.dt.bfloat16, kind="ExternalInput")
    wd = nc.dram_tensor("wd", [P, NB * HSH], mybir.dt.float32, kind="ExternalInput")
    fw = nc.dram_tensor("fw", [F, SH], mybir.dt.bfloat16, kind="ExternalInput")
    d8 = nc.dram_tensor("d8", [P, 4 * B], mybir.dt.float8e4, kind="ExternalInput")
    f8 = nc.dram_tensor("f8", [P, 2 * 4 * 512], mybir.dt.float8e4,
                        kind="ExternalInput")
    partial = nc.dram_tensor("partial", [B, 2 * C], mybir.dt.float32, kind="ExternalOutput")

    with tile.TileContext(nc) as tc:
        with (
            tc.tile_pool(name="const", bufs=1) as cp,
            tc.tile_pool(name="gsp", bufs=2) as gsp,
            tc.tile_pool(name="ps", bufs=2, space="PSUM") as ps,
        ):
            # DMAs in consumption order; first-round operands arrive as
            # small tiles for a fast PE ramp, the rest as big 4KB-row slabs.
            dra = [None] * 4    # dateT[fc], b-columns 0:256 (t 0..1)
            dmid = [None] * 4   # dateT[fc], b-columns 256:1024 (t 2..7)
            drb = [None] * 4    # dateT[fc], b-columns 1024:2048 (t 8..15)
            fw0 = [[None] * 4 for _ in range(4)]   # q0 per-slice tiles [fc][i]
            fwq = [[None] * 4 for _ in range(NQ)]  # q1..q3 slabs [q][fc]
            # first-round operands issue on both HWDGE queues (SP + ACT)
            # so the serial per-DMA issue cost doesn't gate the PE ramp
            for fc in range(4):
                dra[fc] = cp.tile([P, 2 * P], mybir.dt.bfloat16,
                                  name=f"dra{fc}")
                nc.sync.dma_start(dra[fc][:], dateT[fc * P:(fc + 1) * P,
                                                    0:2 * P])
                fw0[fc][0] = cp.tile([P, 512], mybir.dt.bfloat16,
                                     name=f"fw0_{fc}_0")
                nc.scalar.dma_start(fw0[fc][0][:],
                                    fw[fc * P:(fc + 1) * P, 0:512])
            wdp0 = cp.tile([P, 4 * HSH], mybir.dt.float32, name="wdp0")
            nc.scalar.dma_start(wdp0[:], wd[:, 0:4 * HSH])
            for i in range(1, 4):
                for fc in range(4):
                    fw0[fc][i] = cp.tile([P, 512], mybir.dt.bfloat16,
                                         name=f"fw0_{fc}_{i}")
                    nc.sync.dma_start(
                        fw0[fc][i][:],
                        fw[fc * P:(fc + 1) * P, i * 512:(i + 1) * 512])
            for fc in range(4):
                dmid[fc] = cp.tile([P, 6 * P], mybir.dt.bfloat16,
                                   name=f"dmid{fc}")
                nc.sync.dma_start(dmid[fc][:], dateT[fc * P:(fc + 1) * P,
                                                     2 * P:B // 2])
            wdp1 = cp.tile([P, 12 * HSH], mybir.dt.float32, name="wdp1")
            nc.sync.dma_start(wdp1[:], wd[:, 4 * HSH:])
            for fc in range(4):
                drb[fc] = cp.tile([P, B // 2], mybir.dt.bfloat16,
                                  name=f"drb{fc}")
                nc.sync.dma_start(drb[fc][:], dateT[fc * P:(fc + 1) * P,
                                                    B // 2:B])
            for q in range(1, NQ):
                for fc in range(4):
                    fwq[q][fc] = cp.tile([P, 4 * 512], mybir.dt.bfloat16,
                                         name=f"fw{q}_{fc}")
                    nc.sync.dma_start(
                        fwq[q][fc][:],
                        fw[fc * P:(fc + 1) * P, q * 2048:(q + 1) * 2048])
            d8t = cp.tile([P, 4, B], mybir.dt.float8e4, name="d8t")
            nc.sync.dma_start(d8t[:], d8[:, :])
            f8t = []
            for j in range(2):
                ft = cp.tile([P, 4, 512], mybir.dt.float8e4, name=f"f8t{j}")
                nc.sync.dma_start(
                    ft[:], f8[:, j * 2048:(j + 1) * 2048])
                f8t.append(ft)
            accE, accO = [], []
            for t in range(NB):
                a_t = cp.tile([P, C], mybir.dt.float32, name=f"accE{t}")
                nc.any.memset(a_t[:], 0.0)
                accE.append(a_t)
                b_t = cp.tile([P, C], mybir.dt.float32, name=f"accO{t}")
                nc.any.memset(b_t[:], 0.0)
                accO.append(b_t)

            for q in range(NQ):
                for t in range(NB):
                    if t < 2:
                        dcol, tc_ = dra, t
                    elif t < 8:
                        dcol, tc_ = dmid, t - 2
                    else:
                        dcol, tc_ = drb, t - 8
                    g = [ps.tile([P, 512], mybir.dt.float32, name="g",
                                 tag=f"g{i}") for i in range(4)]
                    # slice-major: each PSUM bank's accumulation completes
                    # early in the round, so its copy+STT chain overlaps the
                    # remaining matmuls instead of spilling into the next
                    # round (LDWEIGHTS is emitted per-matmul either way)
                    for i in range(4):
                        if q == NQ - 1 and i >= 2:
                            # fp8 DoubleRow: K=256 per pass, 2 passes
                            for ks in (0, 2):
                                nc.tensor.matmul(
                                    g[i][:],
                                    d8t[:, ks:ks + 2, t * P:(t + 1) * P],
                                    f8t[i - 2][:, ks:ks + 2, :],
                                    start=(ks == 0), stop=(ks == 2),
                                    perf_mode=mybir.MatmulPerfMode.DoubleRow)
                        else:
                            for fc in range(4):
                                rhs = (fw0[fc][i][:] if q == 0 else
                                       fwq[q][fc][:, i * 512:(i + 1) * 512])
                                nc.tensor.matmul(
                                    g[i][:],
                                    dcol[fc][:, tc_ * P:(tc_ + 1) * P],
                                    rhs, start=(fc == 0), stop=(fc == 3))
                    for i in range(4):
                        acc = accE[t] if i % 2 == 0 else accO[t]
                        gs_i = gsp.tile([P, 512], mybir.dt.float32,
                                        name="gs", tag=f"gs{i}")
                        nc.scalar.copy(gs_i[:], g[i][:])
                        for l in range(4):
                            if t < 4:
                                wdt, hcol = wdp0, t * HSH + 16 * q + 4 * i + l
                            else:
                                wdt = wdp1
                                hcol = (t - 4) * HSH + 16 * q + 4 * i + l
                            nc.vector.scalar_tensor_tensor(
                                out=acc[:],
                                in0=gs_i[:, l * C:(l + 1) * C],
                                scalar=wdt[:, hcol:hcol + 1],
                                in1=acc[:],
                                op0=mybir.AluOpType.mult,
                                op1=mybir.AluOpType.add,
                            )
                    if q == NQ - 1:
                        # host adds the two halves; E-half DMA overlaps the
                        # O-chain's final STTs
                        nc.sync.dma_start(partial[t * P:(t + 1) * P, 0:C],
                                          accE[t][:])
                        nc.sync.dma_start(partial[t * P:(t + 1) * P, C:2 * C],
                                          accO[t][:])
    nc.finalize()
    return nc


def _host_phase1(idx, date, train_dates, mem, train_nns, pos_w, pos_b,
                 field_b, adapt_w, adapt_b):
    refs = train_nns[idx]                                   # [B, K]
    pos_d = date @ pos_w + pos_b                            # [B, H]
    pos_r = (train_dates[refs.reshape(-1)] @ pos_w + pos_b).reshape(B, K, H)
    diff = pos_d[:, None, :] - pos_r                        # [B, K, H]
    norm = np.sqrt((diff * diff).sum(-1))                   # [B, K]
    m = norm.min(axis=1, keepdims=True)
    e = np.exp(m - norm)
    dist = e / e.sum(axis=1, keepdims=True)                 # [B, K]
    wdiff = np.einsum("bk,bkh->bh", dist, diff).astype(np.float32)
    wmem = np.einsum("bk,bkc->bc", dist, mem[refs]).astype(np.float32)
    const = wmem @ adapt_w + adapt_b + wdiff @ field_b.reshape(H, C)
    return wdiff, const.astype(np.float32)


def kernel(idx, date, train_dates, mem, train_nns, pos_w, pos_b, field_w,
           field_b, adapt_w, adapt_b):
    global _NC
    from concourse.bass_utils import run_bass_kernel_spmd

    idx = np.asarray(idx)
    date = np.asarray(date, dtype=np.float32)
    train_dates = np.asarray(train_dates, dtype=np.float32)
    mem = np.asarray(mem, dtype=np.float32)
    train_nns = np.asarray(train_nns)
    pos_w = np.asarray(pos_w, dtype=np.float32)
    pos_b = np.asarray(pos_b, dtype=np.float32)
    field_w = np.asarray(field_w, dtype=np.float32)
    field_b = np.asarray(field_b, dtype=np.float32)
    adapt_w = np.asarray(adapt_w, dtype=np.float32)
    adapt_b = np.asarray(adapt_b, dtype=np.float32)

    wdiff, const = _host_phase1(idx, date, train_dates, mem, train_nns,
                                pos_w, pos_b, field_b, adapt_w, adapt_b)

    if _NC is None:
        _NC = _build()
    dateT16 = np.ascontiguousarray(date.T).astype(ml_dtypes.bfloat16)
    fw16 = field_w.astype(ml_dtypes.bfloat16)
    SD, SF = 16.0, 64.0
    dT = date.T                                              # [F, B]
    d8 = np.clip(dT * SD, -240, 240).astype(ml_dtypes.float8_e4m3)
    d8p = np.ascontiguousarray(
        d8.reshape(4, P, B).transpose(1, 0, 2).reshape(P, 4 * B))
    in_maps = []
    for i in range(NCORES):
        wds = wdiff[:, i * HSH:(i + 1) * HSH].copy()         # [B, HSH]
        wds[:, 56:64] *= 1.0 / (SD * SF)                     # DR slices 14,15
        wdp = np.ascontiguousarray(
            wds.reshape(NB, P, HSH).transpose(1, 0, 2).reshape(P, NB * HSH))
        fshard = field_w[:, i * SH:(i + 1) * SH]
        f8 = np.clip(fshard[:, 14 * 512:] * SF, -240,
                     240).astype(ml_dtypes.float8_e4m3)      # [F, 1024]
        f8p = np.ascontiguousarray(
            f8.reshape(4, P, 2, 512).transpose(2, 1, 0, 3)
            .reshape(2, P, 4 * 512).transpose(1, 0, 2).reshape(P, 2 * 4 * 512))
        in_maps.append({
            "dateT": dateT16,
            "wd": wdp,
            "fw": np.ascontiguousarray(fw16[:, i * SH:(i + 1) * SH]),
            "d8": d8p,
            "f8": f8p,
        })
    res = run_bass_kernel_spmd(_NC, in_maps, core_ids=list(range(NCORES)))
    grad_term = np.zeros((B, C), dtype=np.float32)
    for i in range(NCORES):
        p = res.results[i]["partial"]
        grad_term += p[:, :C] + p[:, C:]
    return (const + grad_term).astype(np.float32)


# revision 8
# speedup vs baseline: 1.0826x; 1.0431x over previous
"""TRN2 Bass kernel for nn_MAD_4612794876395 (retrieval_knn) — v2.

Math identical to baseline: with dist = softmax_k(-||pos_d - pos_r||) and
sum_k dist = 1, the reference collapses to
    out[b,c] = wmem@adapt_w + adapt_b + wdiff@field_b.reshape(H,C)
             + sum_h wdiff[b,h] * (date@field_w)[b, h*C+c]
The 137-GFLOP grad term runs on 8 NeuronCores, tensor-parallel over
field_w's 65536 columns (64 h per core).

v2 vs baseline (which was DVE-bound at ~300us):
  - operands pre-converted to bf16 on host (half DMA, FWL weight loads)
  - loop order (slice-quad, b-tile): stationary date chunk reused across
    4 matmuls -> LDWEIGHTS amortized 4x; 8 PSUM banks ping-pong
  - h-contraction: one ACT copy [128,512] PSUM->SBUF per slice, then 4
    SBUF-only DVE scalar_tensor_tensor ops (DVE busy 264us -> 200us) into
    two independent accumulator chains (even/odd slices); host adds halves
  - few, large DMAs (4KB rows) emitted in consumption order; wd packed
    on host into one [128, NB*HSH] tile
"""
import sys

sys.path.insert(0, "/opt/trn_rl_repo")

import numpy as np
import ml_dtypes

N_DATA, F, H, C, K, B = 100000, 512, 512, 128, 8, 2048
NCORES = 8
HSH = H // NCORES          # 64 h-values per core
SH = HSH * C               # 8192 field_w cols per core
P = 128
NB = B // P                # 16 b-tiles
NS = SH // 512             # 16 n-slices of 512 cols (4 h each)
NQ = NS // 4               # 4 slice-quads

_NC = None


def _build():
    import concourse.mybir as mybir
    import concourse.tile as tile
    from concourse import bacc

    nc = bacc.Bacc(None, target_bir_lowering=False, debug=False)
    dateT = nc.dram_tensor("dateT", [F, B], mybir.dt.bfloat16, kind="ExternalInput")
    wd = nc.dram_tensor("wd", [P, NB * HSH], mybir.dt.float32, kind="ExternalInput")
    fw = nc.dram_tensor("fw", [F, SH], mybir.dt.bfloat16, kind="ExternalInput")
    d8 = nc.dram_tensor("d8", [P, 4 * B], mybir.dt.float8e4, kind="ExternalInput")
    f8 = nc.dram_tensor("f8", [P, 2 * 4 * 512], mybir.dt.float8e4,
                        kind="ExternalInput")
    partial = nc.dram_tensor("partial", [B, 2 * C], mybir.dt.float32, kind="ExternalOutput")

    with tile.TileContext(nc) as tc:
        with (
            tc.tile_pool(name="const", bufs=1) as cp,
            tc.tile_pool(name="gsp", bufs=2) as gsp,
            tc.tile_pool(name="ps", bufs=2, space="PSUM") as ps,
        ):
            # DMAs in consumption order; first-round operands arrive as
            # small tiles for a fast PE ramp, the rest as big 4KB-row slabs.
            dra = [None] * 4    # dateT[fc], b-columns 0:256 (t 0..1)
            dmid = [None] * 4   # dateT[fc], b-columns 256:1024 (t 2..7)
            drb = [None] * 4    # dateT[fc], b-columns 1024:2048 (t 8..15)
            fw0 = [[None] * 4 for _ in range(4)]   # q0 per-slice tiles [fc][i]
            fwq = [[None] * 4 for _ in range(NQ)]  # q1..q3 slabs [q][fc]
            # first-round operands issue on both HWDGE queues (SP + ACT)
            # so the serial per-DMA issue cost doesn't gate the PE ramp
            for fc in range(4):
                dra[fc] = cp.tile([P, 2 * P], mybir.dt.bfloat16,
                                  name=f"dra{fc}")
                nc.sync.dma_start(dra[fc][:], dateT[fc * P:(fc + 1) * P,
                                                    0:2 * P])
                fw0[fc][0] = cp.tile([P, 512], mybir.dt.bfloat16,
                                     name=f"fw0_{fc}_0")
                nc.scalar.dma_start(fw0[fc][0][:],
                                    fw[fc * P:(fc + 1) * P, 0:512])
            wdp0 = cp.tile([P, 4 * HSH], mybir.dt.float32, name="wdp0")
            nc.scalar.dma_start(wdp0[:], wd[:, 0:4 * HSH])
            for i in range(1, 4):
                for fc in range(4):
                    fw0[fc][i] = cp.tile([P, 512], mybir.dt.bfloat16,
                                         name=f"fw0_{fc}_{i}")
                    nc.sync.dma_start(
                        fw0[fc][i][:],
                        fw[fc * P:(fc + 1) * P, i * 512:(i + 1) * 512])
            for fc in range(4):
                dmid[fc] = cp.tile([P, 6 * P], mybir.dt.bfloat16,
                                   name=f"dmid{fc}")
                nc.sync.dma_start(dmid[fc][:], dateT[fc * P:(fc + 1) * P,
                                                     2 * P:B // 2])
            wdp1 = cp.tile([P, 12 * HSH], mybir.dt.float32, name="wdp1")
            nc.sync.dma_start(wdp1[:], wd[:, 4 * HSH:])
            for fc in range(4):
                drb[fc] = cp.tile([P, B // 2], mybir.dt.bfloat16,
                                  name=f"drb{fc}")
                nc.sync.dma_start(drb[fc][:], dateT[fc * P:(fc + 1) * P,
                                                    B // 2:B])
            for q in range(1, NQ):
                for fc in range(4):
                    fwq[q][fc] = cp.tile([P, 4 * 512], mybir.dt.bfloat16,
                                         name=f"fw{q}_{fc}")
                    nc.sync.dma_start(
                        fwq[q][fc][:],
                        fw[fc * P:(fc + 1) * P, q * 2048:(q + 1) * 2048])
            d8t = cp.tile([P, 4, B], mybir.dt.float8e4, name="d8t")
            nc.sync.dma_start(d8t[:], d8[:, :])
            f8t = []
            for j in range(2):
                ft = cp.tile([P, 4, 512], mybir.dt.float8e4, name=f"f8t{j}")
                nc.sync.dma_start(
                    ft[:], f8[:, j * 2048:(j + 1) * 2048])
                f8t.append(ft)
            accE, accO = [], []
            for t in range(NB):
                a_t = cp.tile([P, C], mybir.dt.float32, name=f"accE{t}")
                nc.any.memset(a_t[:], 0.0)
                accE.append(a_t)
                b_t = cp.tile([P, C], mybir.dt.float32, name=f"accO{t}")
                nc.any.memset(b_t[:], 0.0)
                accO.append(b_t)

            # q0 first (fw q1-q3 still streaming), then interleave the
            # remaining quads per b-tile: the fast fp8 q3 rounds supply DVE
            # work into the bf16 rounds' starvation slack
            schedule = [(0, t) for t in range(NB)]
            for t in range(NB):
                schedule += [(1, t), (2, t), (3, t)]
            for q, t in schedule:
                if True:
                    if t < 2:
                        dcol, tc_ = dra, t
                    elif t < 8:
                        dcol, tc_ = dmid, t - 2
                    else:
                        dcol, tc_ = drb, t - 8
                    g = [ps.tile([P, 512], mybir.dt.float32, name="g",
                                 tag=f"g{i}") for i in range(4)]
                    # slice-major: each PSUM bank's accumulation completes
                    # early in the round, so its copy+STT chain overlaps the
                    # remaining matmuls instead of spilling into the next
                    # round (LDWEIGHTS is emitted per-matmul either way)
                    for i in range(4):
                        if q == NQ - 1 and i >= 2:
                            # fp8 DoubleRow: K=256 per pass, 2 passes
                            for ks in (0, 2):
                                nc.tensor.matmul(
                                    g[i][:],
                                    d8t[:, ks:ks + 2, t * P:(t + 1) * P],
                                    f8t[i - 2][:, ks:ks + 2, :],
                                    start=(ks == 0), stop=(ks == 2),
                                    perf_mode=mybir.MatmulPerfMode.DoubleRow)
                        else:
                            for fc in range(4):
                                rhs = (fw0[fc][i][:] if q == 0 else
                                       fwq[q][fc][:, i * 512:(i + 1) * 512])
                                nc.tensor.matmul(
                                    g[i][:],
                                    dcol[fc][:, tc_ * P:(tc_ + 1) * P],
                                    rhs, start=(fc == 0), stop=(fc == 3))
                    for i in range(4):
                        acc = accE[t] if i % 2 == 0 else accO[t]
                        gs_i = gsp.tile([P, 512], mybir.dt.float32,
                                        name="gs", tag=f"gs{i}")
                        nc.scalar.copy(gs_i[:], g[i][:])
                        for l in range(4):
                            if t < 4:
                                wdt, hcol = wdp0, t * HSH + 16 * q + 4 * i + l
                            else:
                                wdt = wdp1
                                hcol = (t - 4) * HSH + 16 * q + 4 * i + l
                            nc.vector.scalar_tensor_tensor(
                                out=acc[:],
                                in0=gs_i[:, l * C:(l + 1) * C],
                                scalar=wdt[:, hcol:hcol + 1],
                                in1=acc[:],
                                op0=mybir.AluOpType.mult,
                                op1=mybir.AluOpType.add,
                            )
                    if q == NQ - 1:
                        # host adds the two halves; E-half DMA overlaps the
                        # O-chain's final STTs
                        nc.sync.dma_start(partial[t * P:(t + 1) * P, 0:C],
                                          accE[t][:])
                        nc.sync.dma_start(partial[t * P:(t + 1) * P, C:2 * C],
                                          accO[t][:])
    nc.finalize()
    return nc


def _host_phase1(idx, date, train_dates, mem, train_nns, pos_w, pos_b,
                 field_b, adapt_w, adapt_b):
    refs = train_nns[idx]                                   # [B, K]
    pos_d = date @ pos_w + pos_b                            # [B, H]
    pos_r = (train_dates[refs.reshape(-1)] @ pos_w + pos_b).reshape(B, K, H)
    diff = pos_d[:, None, :] - pos_r                        # [B, K, H]
    norm = np.sqrt((diff * diff).sum(-1))                   # [B, K]
    m = norm.min(axis=1, keepdims=True)
    e = np.exp(m - norm)
    dist = e / e.sum(axis=1, keepdims=True)                 # [B, K]
    wdiff = np.einsum("bk,bkh->bh", dist, diff).astype(np.float32)
    wmem = np.einsum("bk,bkc->bc", dist, mem[refs]).astype(np.float32)
    const = wmem @ adapt_w + adapt_b + wdiff @ field_b.reshape(H, C)
    return wdiff, const.astype(np.float32)


def kernel(idx, date, train_dates, mem, train_nns, pos_w, pos_b, field_w,
           field_b, adapt_w, adapt_b):
    global _NC
    from concourse.bass_utils import run_bass_kernel_spmd

    idx = np.asarray(idx)
    date = np.asarray(date, dtype=np.float32)
    train_dates = np.asarray(train_dates, dtype=np.float32)
    mem = np.asarray(mem, dtype=np.float32)
    train_nns = np.asarray(train_nns)
    pos_w = np.asarray(pos_w, dtype=np.float32)
    pos_b = np.asarray(pos_b, dtype=np.float32)
    field_w = np.asarray(field_w, dtype=np.float32)
    field_b = np.asarray(field_b, dtype=np.float32)
    adapt_w = np.asarray(adapt_w, dtype=np.float32)
    adapt_b = np.asarray(adapt_b, dtype=np.float32)

    wdiff, const = _host_phase1(idx, date, train_dates, mem, train_nns,
                                pos_w, pos_b, field_b, adapt_w, adapt_b)

    if _NC is None:
        _NC = _build()
    dateT16 = np.ascontiguousarray(date.T).astype(ml_dtypes.bfloat16)
    fw16 = field_w.astype(ml_dtypes.bfloat16)
    SD, SF = 16.0, 64.0
    dT = date.T                                              # [F, B]
    d8 = np.clip(dT * SD, -240, 240).astype(ml_dtypes.float8_e4m3)
    d8p = np.ascontiguousarray(
        d8.reshape(4, P, B).transpose(1, 0, 2).reshape(P, 4 * B))
    in_maps = []
    for i in range(NCORES):
        wds = wdiff[:, i * HSH:(i + 1) * HSH].copy()         # [B, HSH]
        wds[:, 56:64] *= 1.0 / (SD * SF)                     # DR slices 14,15
        wdp = np.ascontiguousarray(
            wds.reshape(NB, P, HSH).transpose(1, 0, 2).reshape(P, NB * HSH))
        fshard = field_w[:, i * SH:(i + 1) * SH]
        f8 = np.clip(fshard[:, 14 * 512:] * SF, -240,
                     240).astype(ml_dtypes.float8_e4m3)      # [F, 1024]
        f8p = np.ascontiguousarray(
            f8.reshape(4, P, 2, 512).transpose(2, 1, 0, 3)
            .reshape(2, P, 4 * 512).transpose(1, 0, 2).reshape(P, 2 * 4 * 512))
        in_maps.append({
            "dateT": dateT16,
            "wd": wdp,
            "fw": np.ascontiguousarray(fw16[:, i * SH:(i + 1) * SH]),
            "d8": d8p,
            "f8": f8p,
        })
    res = run_bass_kernel_spmd(_NC, in_maps, core_ids=list(range(NCORES)))
    grad_term = np.zeros((B, C), dtype=np.float32)
    for i in range(NCORES):
        p = res.results[i]["partial"]
        grad_term += p[:, :C] + p[:, C:]
    return (const + grad_term).astype(np.float32)
